# revision 17
# baseline (speedup 1.0000x reference)
"""Trainium2 Bass kernel for nn_CFTL_60327110640070.

out = x + ifft_c( fused(fft_c(mean_hw(x)), g@W1.T+b1, g@W2.T+b2) )  broadcast over HW

Strategy (pure data parallel, 8 cores, 2 samples each):
  pass 1: stream x tiles [128ch, FREE] from HBM (SP/HWDGE), DVE reduce-sum -> g sums
  stats : FFT/IFFT as 128x128-block matmuls against cos/-sin DFT matrices (PE),
          tiny elementwise chain on [128,4] tiles (DVE/ACT)
  pass 2: re-stream x tiles, DVE per-channel scalar add, store via GPSIMD/SWDGE

Raw bass (no Tile): this walrus build only allows one embedded sync-wait per
DMA pseudo-instruction, so all waits are standalone wait_ge on the issuing
engine and DMAs carry only their completion-sem update.

All DFT/weight matrices are pre-transposed/pre-scaled on host so no on-device
transposes are needed (cos/-sin DFT matrices are symmetric).
"""

import sys
from contextlib import ExitStack

for _p in ("/opt/trn_rl_repo", "/root/.axon_site/_ro/trn_rl_repo"):
    if _p not in sys.path:
        sys.path.append(_p)

import numpy as np

import concourse.bass as bass
from concourse import mybir
from concourse.bass_utils import run_bass_kernel_spmd

# Problem geometry (hardcoded per contract)
N, C, H, W = 16, 512, 128, 128
HW = H * W
NCORES = 8
NS = N // NCORES          # samples per core = 2
P = 128                   # SBUF partitions
G = C // P                # channel groups = 4
FREE = 8192               # free-dim tile size for streaming x
NB_IN = 3                 # load ring buffers
NB_OUT = 2                # store ring buffers

_FP32 = mybir.dt.float32
_AF = mybir.ActivationFunctionType


def _build_program(free=FREE, hw=HW, nb_in=NB_IN, nb_out=NB_OUT) -> bass.Bass:
    nhalf = hw // free           # tiles per (sample, group)
    tps = G * nhalf              # x tiles per sample
    n_x = NS * tps               # x tiles per pass
    n_const = 5

    nc = bass.Bass()

    x_in = nc.dram_tensor("x", [NS, C, hw], _FP32, kind="ExternalInput")
    x_out = nc.dram_tensor("out", [NS, C, hw], _FP32, kind="ExternalOutput")
    # host pre-layouts: [p, g, k] with row index c = g*128+p
    cos_d = nc.dram_tensor("cosm", [P, G, C], _FP32, kind="ExternalInput")
    sin_d = nc.dram_tensor("sinn", [P, G, C], _FP32, kind="ExternalInput")
    w1_d = nc.dram_tensor("w1t", [P, G, C], _FP32, kind="ExternalInput")
    w2_d = nc.dram_tensor("w2t", [P, G, C], _FP32, kind="ExternalInput")
    b_d = nc.dram_tensor("bvec", [P, 2, G], _FP32, kind="ExternalInput")

    def x_ap(dram, j):
        """AP of x tile j (load order: pass index-free; j in [0, n_x))."""
        s, r = divmod(j, tps)
        cg, h = divmod(r, nhalf)
        return dram[s, cg * P:(cg + 1) * P, h * free:(h + 1) * free]

    with ExitStack() as ctx:
        sb = lambda shape, name: ctx.enter_context(
            nc.sbuf_tensor(name, shape, _FP32)
        )
        ps = lambda shape, name: ctx.enter_context(
            nc.psum_tensor(name, shape, _FP32)
        )
        sem = lambda name: ctx.enter_context(nc.semaphore(name))

        cos_sb = sb([P, G, C], "cos_sb")
        sin_sb = sb([P, G, C], "sin_sb")
        w1_sb = sb([P, G, C], "w1_sb")
        w2_sb = sb([P, G, C], "w2_sb")
        b_sb = sb([P, 2, G], "b_sb")
        halfpi = sb([P, 1], "halfpi")

        xt = [sb([P, free], f"xt{i}") for i in range(nb_in)]
        yt = [sb([P, free], f"yt{i}") for i in range(nb_out)]

        gsum = [sb([P, G, nhalf], f"gsum{s}") for s in range(NS)]
        gcol = [sb([P, G], f"gcol{s}") for s in range(NS)]
        fr = [sb([P, G], f"fr{s}") for s in range(NS)]
        fi = [sb([P, G], f"fi{s}") for s in range(NS)]
        z12 = [sb([P, 2, G], f"z12_{s}") for s in range(NS)]
        r2 = [sb([P, 2, G], f"r2_{s}") for s in range(NS)]
        s12 = [sb([P, 2, G], f"s12_{s}") for s in range(NS)]
        u0 = [sb([P, G], f"u0_{s}") for s in range(NS)]
        u1 = [sb([P, G], f"u1_{s}") for s in range(NS)]
        amp = [sb([P, G], f"amp{s}") for s in range(NS)]
        apr = [sb([P, G], f"apr{s}") for s in range(NS)]
        ppr = [sb([P, G], f"ppr{s}") for s in range(NS)]
        cosp = [sb([P, G], f"cosp{s}") for s in range(NS)]
        sinp = [sb([P, G], f"sinp{s}") for s in range(NS)]
        zr = [sb([P, G], f"zr{s}") for s in range(NS)]
        zi = [sb([P, G], f"zi{s}") for s in range(NS)]
        xi = [sb([P, G], f"xi{s}") for s in range(NS)]

        fwd_ps = [ps([P, 4, G], f"fwd_ps{s}") for s in range(NS)]
        xi_ps = [ps([P, G], f"xi_ps{s}") for s in range(NS)]

        # Per-ring-slot DMA completion sems: ring flow control guarantees at
        # most one in-flight DMA per slot sem, so waited values are always
        # full totals (required -- partial cumulative waits race with the
        # 16 per-engine micro-increments of in-flight DMAs).
        ld_slot = [sem(f"ld_slot{i}") for i in range(nb_in)]
        st_slot = [sem(f"st_slot{i}") for i in range(nb_out)]
        sem_cst = sem("sem_cst")    # const load completions (+16 each)
        sem_cons = sem("sem_cons")  # DVE consumed an xt tile (+1, load order)
        sem_dve = sem("sem_dve")    # DVE stats milestones
        sem_act = sem("sem_act")    # ACT stats milestones
        sem_pe = sem("sem_pe")      # PE matmul groups

        # Planned sem values after each named op, precomputed so any engine
        # stream can reference any other's milestones regardless of emission
        # order. Emitters assert against these as they bump.
        plan = {"memset": 1}
        for s in range(NS):
            plan[f"gcol{s}"] = 2 + s
            base = 1 + NS + 9 * s  # dve count before sample s stats
            plan[f"z12_{s}"] = base + 1
            plan[f"s12_{s}"] = base + 2
            plan[f"u1m_{s}"] = base + 4
            plan[f"u0_{s}"] = base + 5
            plan[f"apr_{s}"] = base + 6
            plan[f"ppr_{s}"] = base + 7
            plan[f"zi_{s}"] = base + 9
            plan[f"fi_ev_{s}"] = 7 * s + 2
            plan[f"r2_{s}"] = 7 * s + 3
            plan[f"amp_{s}"] = 7 * s + 4
            plan[f"sinp_{s}"] = 7 * s + 6
            plan[f"xi_ev_{s}"] = 7 * s + 7

        dve_v = {"n": 0}
        act_v = {"n": 0}

        with nc.Block() as block:

            @block.vector
            def _(dve):
                nv = dve_v

                def bump(tag=None):
                    nv["n"] += 1
                    if tag:
                        assert plan[tag] == nv["n"], (tag, plan[tag], nv["n"])
                    return nv["n"]

                nc.vector.memset(halfpi[:], float(np.pi / 2)).then_inc(sem_dve, 1)
                bump("memset")
                cons = 0
                # ---- pass 1 reduces ----
                for s in range(NS):
                    for r in range(tps):
                        j = s * tps + r
                        cg, h = divmod(r, nhalf)
                        dve.wait_ge(ld_slot[j % nb_in], 16 * (j // nb_in + 1))
                        nc.vector.reduce_sum(
                            gsum[s][:, cg, h:h + 1], xt[j % nb_in][:],
                            axis=mybir.AxisListType.X,
                        ).then_inc(sem_cons, 1)
                        cons += 1
                    dve.wait_ge(sem_cons, (s + 1) * tps)  # own gsum writes done
                    nc.vector.reduce_sum(
                        gcol[s][:], gsum[s][:], axis=mybir.AxisListType.X
                    ).then_inc(sem_dve, 1)
                    bump(f"gcol{s}")
                # ---- stats (DVE part) ----
                for s in range(NS):
                    dve.wait_ge(sem_pe, s + 1)  # fwd matmuls of s done
                    if s == 0:
                        dve.wait_ge(sem_cst, 16 * n_const)  # b_sb resident
                    nc.vector.tensor_add(
                        z12[s][:], fwd_ps[s][:, 2:4, :], b_sb[:]
                    ).then_inc(sem_dve, 1)
                    bump(f"z12_{s}")
                    # leaky_relu(z) = z + 0.99*relu(-z)
                    dve.wait_ge(sem_act, plan[f"r2_{s}"])
                    dve.wait_ge(sem_dve, plan[f"z12_{s}"])  # self RAW
                    nc.vector.scalar_tensor_tensor(
                        out=s12[s][:], in0=r2[s][:], scalar=0.99, in1=z12[s][:],
                        op0=mybir.AluOpType.mult, op1=mybir.AluOpType.add,
                    ).then_inc(sem_dve, 1)
                    bump(f"s12_{s}")
                    dve.wait_ge(sem_act, plan[f"fi_ev_{s}"])
                    nc.vector.tensor_mul(u0[s][:], fr[s][:], fr[s][:]).then_inc(
                        sem_dve, 1
                    )
                    bump()
                    nc.vector.tensor_mul(u1[s][:], fi[s][:], fi[s][:]).then_inc(
                        sem_dve, 1
                    )
                    bump(f"u1m_{s}")
                    dve.wait_ge(sem_dve, plan[f"u1m_{s}"])  # self RAW u0/u1
                    nc.vector.tensor_add(u0[s][:], u0[s][:], u1[s][:]).then_inc(
                        sem_dve, 1
                    )
                    bump(f"u0_{s}")
                    dve.wait_ge(sem_act, plan[f"amp_{s}"])
                    dve.wait_ge(sem_dve, plan[f"s12_{s}"])  # self RAW
                    nc.vector.tensor_mul(apr[s][:], s12[s][:, 0, :], amp[s][:]).then_inc(
                        sem_dve, 1
                    )
                    bump(f"apr_{s}")
                    nc.vector.tensor_mul(
                        ppr[s][:], s12[s][:, 1, :], fi[s][:]
                    ).then_inc(sem_dve, 1)
                    bump(f"ppr_{s}")
                    dve.wait_ge(sem_act, plan[f"sinp_{s}"])
                    dve.wait_ge(sem_dve, plan[f"apr_{s}"])  # self RAW
                    nc.vector.tensor_mul(zr[s][:], apr[s][:], cosp[s][:]).then_inc(
                        sem_dve, 1
                    )
                    bump()
                    nc.vector.tensor_mul(zi[s][:], apr[s][:], sinp[s][:]).then_inc(
                        sem_dve, 1
                    )
                    bump(f"zi_{s}")
                # ---- pass 2 adds ----
                for m in range(n_x):
                    s, r = divmod(m, tps)
                    cg = r // nhalf
                    j = n_x + m  # global load index
                    if r == 0:
                        dve.wait_ge(sem_act, plan[f"xi_ev_{s}"])
                    dve.wait_ge(ld_slot[j % nb_in], 16 * (j // nb_in + 1))
                    if m >= nb_out:
                        dve.wait_ge(
                            st_slot[m % nb_out], 16 * ((m - nb_out) // nb_out + 1)
                        )
                    nc.vector.tensor_scalar_add(
                        yt[m % nb_out][:], xt[j % nb_in][:], xi[s][:, cg:cg + 1]
                    ).then_inc(sem_cons, 1)
                    cons += 1

            @block.scalar
            def _(act):
                nv = act_v

                def bump(tag=None):
                    nv["n"] += 1
                    if tag:
                        assert plan[tag] == nv["n"], (tag, plan[tag], nv["n"])
                    return nv["n"]

                act.wait_ge(sem_dve, plan["memset"])
                for s in range(NS):
                    act.wait_ge(sem_pe, s + 1)
                    nc.scalar.mul(fr[s][:], fwd_ps[s][:, 0, :], 1.0 / hw)
                    bump()
                    nc.scalar.mul(fi[s][:], fwd_ps[s][:, 1, :], 1.0 / hw).then_inc(
                        sem_act, 2
                    )
                    bump(f"fi_ev_{s}")
                    act.wait_ge(sem_dve, plan[f"z12_{s}"])
                    nc.scalar.activation(
                        r2[s][:], z12[s][:], _AF.Relu, scale=-1.0
                    ).then_inc(sem_act, 1)
                    bump(f"r2_{s}")
                    act.wait_ge(sem_dve, plan[f"u0_{s}"])
                    nc.scalar.activation(amp[s][:], u0[s][:], _AF.Sqrt).then_inc(
                        sem_act, 1
                    )
                    bump(f"amp_{s}")
                    act.wait_ge(sem_dve, plan[f"ppr_{s}"])
                    nc.scalar.activation(
                        cosp[s][:], ppr[s][:], _AF.Sin, bias=halfpi[:]
                    )
                    bump()
                    nc.scalar.activation(sinp[s][:], ppr[s][:], _AF.Sin).then_inc(
                        sem_act, 2
                    )
                    bump(f"sinp_{s}")
                    act.wait_ge(sem_pe, NS + s + 1)  # inverse matmuls of s done
                    nc.scalar.mul(xi[s][:], xi_ps[s][:], 1.0 / C).then_inc(
                        sem_act, 1
                    )
                    bump(f"xi_ev_{s}")

            @block.tensor
            def _(pe):
                pe.wait_ge(sem_cst, 16 * n_const)  # consts resident
                for s in range(NS):
                    pe.wait_ge(sem_dve, plan[f"gcol{s}"])
                    last = None
                    for t, mat in enumerate((cos_sb, sin_sb, w1_sb, w2_sb)):
                        for kg in range(G):
                            for cg in range(G):
                                last = nc.tensor.matmul(
                                    fwd_ps[s][:, t, kg:kg + 1],
                                    mat[:, cg, kg * P:(kg + 1) * P],
                                    gcol[s][:, cg:cg + 1],
                                    start=(cg == 0),
                                    stop=(cg == G - 1),
                                )
                    last.then_inc(sem_pe, 1)  # value s+1
                for s in range(NS):
                    pe.wait_ge(sem_dve, plan[f"zi_{s}"])
                    last = None
                    for cg in range(G):
                        for kg in range(G):
                            nc.tensor.matmul(
                                xi_ps[s][:, cg:cg + 1],
                                cos_sb[:, kg, cg * P:(cg + 1) * P],
                                zr[s][:, kg:kg + 1],
                                start=(kg == 0),
                                stop=False,
                            )
                            last = nc.tensor.matmul(
                                xi_ps[s][:, cg:cg + 1],
                                sin_sb[:, kg, cg * P:(cg + 1) * P],
                                zi[s][:, kg:kg + 1],
                                start=False,
                                stop=(kg == G - 1),
                            )
                    last.then_inc(sem_pe, 1)  # value NS+s+1

            @block.sync
            def _(sp):
                for dram, sbuf in (
                    (cos_d, cos_sb), (sin_d, sin_sb), (w1_d, w1_sb),
                    (w2_d, w2_sb), (b_d, b_sb),
                ):
                    sp.dma_start(out=sbuf[:], in_=dram[:]).then_inc(sem_cst, 16)
                for j in range(2 * n_x):
                    if j >= nb_in:
                        sp.wait_ge(sem_cons, j - nb_in + 1)
                    sp.dma_start(
                        out=xt[j % nb_in][:], in_=x_ap(x_in, j % n_x)
                    ).then_inc(ld_slot[j % nb_in], 16)

            @block.gpsimd
            def _(gp):
                for m in range(n_x):
                    gp.wait_ge(sem_cons, n_x + m + 1)
                    gp.dma_start(
                        out=x_ap(x_out, m), in_=yt[m % nb_out][:]
                    ).then_inc(st_slot[m % nb_out], 16)

    return nc


_NC_CACHE = None


def _get_program():
    global _NC_CACHE
    if _NC_CACHE is None:
        _NC_CACHE = _build_program()
    return _NC_CACHE


def _host_constants():
    idx = np.arange(C)
    th = (2.0 * np.pi / C) * np.outer(idx, idx)
    cosm = np.cos(th).astype(np.float32)
    sinn = (-np.sin(th)).astype(np.float32)
    # [p, g, k] layout with row c = g*128+p
    to_pgk = lambda m: np.ascontiguousarray(m.reshape(G, P, C).transpose(1, 0, 2))
    return to_pgk(cosm), to_pgk(sinn)


_CONSTS_CACHE = None


def make_in_maps(inputs, hw=HW):
    """Shard + preprocess inputs into 8 per-core input maps."""
    global _CONSTS_CACHE
    if _CONSTS_CACHE is None:
        _CONSTS_CACHE = _host_constants()
    cos_pgk, sin_pgk = _CONSTS_CACHE

    x = np.ascontiguousarray(inputs["x"], dtype=np.float32)
    W1 = np.asarray(inputs["W1"], dtype=np.float32)
    W2 = np.asarray(inputs["W2"], dtype=np.float32)
    b1 = np.asarray(inputs["b1"], dtype=np.float32)
    b2 = np.asarray(inputs["b2"], dtype=np.float32)

    # fold the 1/HW mean normalization into the linear-layer weights
    w1t = np.ascontiguousarray(
        (W1.T / hw).reshape(G, P, C).transpose(1, 0, 2), dtype=np.float32
    )
    w2t = np.ascontiguousarray(
        (W2.T / hw).reshape(G, P, C).transpose(1, 0, 2), dtype=np.float32
    )
    bvec = np.ascontiguousarray(
        np.stack([b1.reshape(G, P), b2.reshape(G, P)]).transpose(2, 0, 1),
        dtype=np.float32,
    )  # [P, 2, G]

    xs = x.reshape(NCORES, NS, C, hw)
    return [
        {
            "x": xs[i],
            "cosm": cos_pgk,
            "sinn": sin_pgk,
            "w1t": w1t,
            "w2t": w2t,
            "bvec": bvec,
        }
        for i in range(NCORES)
    ]


def _run(inputs, trace=False, trace_kwargs=None):
    in_maps = make_in_maps(inputs)
    nc = _get_program()
    res = run_bass_kernel_spmd(
        nc,
        in_maps,
        list(range(NCORES)),
        trace=trace,
        **(trace_kwargs or {}),
    )
    out = np.stack([r["out"] for r in res.results])
    return out.reshape(N, C, H, W).astype(np.float32), res


def kernel(**inputs) -> np.ndarray:
    out, _ = _run(inputs, trace=False)
    return out


# revision 23
# speedup vs baseline: 1.0478x; 1.0478x over previous
"""Trainium2 Bass kernel for nn_CFTL_60327110640070.

out = x + ifft_c( fused(fft_c(mean_hw(x)), g@W1.T+b1, g@W2.T+b2) )  broadcast over HW

Strategy (pure data parallel, 8 cores, 2 samples each):
  pass 1: stream x tiles [128ch, FREE] from HBM (SP/HWDGE), DVE reduce-sum -> g sums
  stats : FFT/IFFT as 128x128-block matmuls against cos/-sin DFT matrices (PE),
          tiny elementwise chain on [128,4] tiles (DVE/ACT)
  pass 2: re-stream x tiles, DVE per-channel scalar add, store via GPSIMD/SWDGE

Raw bass (no Tile): this walrus build only allows one embedded sync-wait per
DMA pseudo-instruction, so all waits are standalone wait_ge on the issuing
engine and DMAs carry only their completion-sem update.

All DFT/weight matrices are pre-transposed/pre-scaled on host so no on-device
transposes are needed (cos/-sin DFT matrices are symmetric).
"""

import sys
from contextlib import ExitStack

for _p in ("/opt/trn_rl_repo", "/root/.axon_site/_ro/trn_rl_repo"):
    if _p not in sys.path:
        sys.path.append(_p)

import numpy as np

import concourse.bass as bass
from concourse import mybir
from concourse.bass_utils import run_bass_kernel_spmd

# Problem geometry (hardcoded per contract)
N, C, H, W = 16, 512, 128, 128
HW = H * W
NCORES = 8
NS = N // NCORES          # samples per core = 2
P = 128                   # SBUF partitions
G = C // P                # channel groups = 4
FREE = 8192               # free-dim tile size for streaming x
NB_IN = 3                 # load ring buffers
NB_OUT = 2                # store ring buffers

_FP32 = mybir.dt.float32
_AF = mybir.ActivationFunctionType


def _build_program(free=FREE, hw=HW, nb_in=NB_IN, nb_out=NB_OUT) -> bass.Bass:
    nhalf = hw // free           # tiles per (sample, group)
    tps = G * nhalf              # x tiles per sample
    n_x = NS * tps               # x tiles per pass
    n_const = 5

    nc = bass.Bass()

    x_in = nc.dram_tensor("x", [NS, C, hw], _FP32, kind="ExternalInput")
    x_out = nc.dram_tensor("out", [NS, C, hw], _FP32, kind="ExternalOutput")
    # host pre-layouts: [p, g, k] with row index c = g*128+p
    cos_d = nc.dram_tensor("cosm", [P, G, C], _FP32, kind="ExternalInput")
    sin_d = nc.dram_tensor("sinn", [P, G, C], _FP32, kind="ExternalInput")
    w1_d = nc.dram_tensor("w1t", [P, G, C], _FP32, kind="ExternalInput")
    w2_d = nc.dram_tensor("w2t", [P, G, C], _FP32, kind="ExternalInput")
    b_d = nc.dram_tensor("bvec", [P, 2, G], _FP32, kind="ExternalInput")

    def x_ap(dram, j):
        """AP of x tile j (load order: pass index-free; j in [0, n_x))."""
        s, r = divmod(j, tps)
        cg, h = divmod(r, nhalf)
        return dram[s, cg * P:(cg + 1) * P, h * free:(h + 1) * free]

    with ExitStack() as ctx:
        sb = lambda shape, name: ctx.enter_context(
            nc.sbuf_tensor(name, shape, _FP32)
        )
        ps = lambda shape, name: ctx.enter_context(
            nc.psum_tensor(name, shape, _FP32)
        )
        sem = lambda name: ctx.enter_context(nc.semaphore(name))

        cos_sb = sb([P, G, C], "cos_sb")
        sin_sb = sb([P, G, C], "sin_sb")
        w1_sb = sb([P, G, C], "w1_sb")
        w2_sb = sb([P, G, C], "w2_sb")
        b_sb = sb([P, 2, G], "b_sb")
        halfpi = sb([P, 1], "halfpi")

        xt = [sb([P, free], f"xt{i}") for i in range(nb_in)]
        yt = [sb([P, free], f"yt{i}") for i in range(nb_out)]

        gsum = [sb([P, G, nhalf], f"gsum{s}") for s in range(NS)]
        gcol = [sb([P, G], f"gcol{s}") for s in range(NS)]
        fr = [sb([P, G], f"fr{s}") for s in range(NS)]
        fi = [sb([P, G], f"fi{s}") for s in range(NS)]
        z12 = [sb([P, 2, G], f"z12_{s}") for s in range(NS)]
        r2 = [sb([P, 2, G], f"r2_{s}") for s in range(NS)]
        s12 = [sb([P, 2, G], f"s12_{s}") for s in range(NS)]
        u0 = [sb([P, G], f"u0_{s}") for s in range(NS)]
        u1 = [sb([P, G], f"u1_{s}") for s in range(NS)]
        amp = [sb([P, G], f"amp{s}") for s in range(NS)]
        apr = [sb([P, G], f"apr{s}") for s in range(NS)]
        ppr = [sb([P, G], f"ppr{s}") for s in range(NS)]
        cosp = [sb([P, G], f"cosp{s}") for s in range(NS)]
        sinp = [sb([P, G], f"sinp{s}") for s in range(NS)]
        zr = [sb([P, G], f"zr{s}") for s in range(NS)]
        zi = [sb([P, G], f"zi{s}") for s in range(NS)]
        xi = [sb([P, G], f"xi{s}") for s in range(NS)]

        fwd_ps = [ps([P, 4, G], f"fwd_ps{s}") for s in range(NS)]
        xi_ps = [ps([P, G], f"xi_ps{s}") for s in range(NS)]

        # Per-ring-slot DMA completion sems: ring flow control guarantees at
        # most one in-flight DMA per slot sem, so waited values are always
        # full totals (required -- partial cumulative waits race with the
        # 16 per-engine micro-increments of in-flight DMAs).
        ld_slot = [sem(f"ld_slot{i}") for i in range(nb_in)]
        st_slot = [sem(f"st_slot{i}") for i in range(nb_out)]
        sem_cst = sem("sem_cst")    # const load completions (+16 each)
        sem_cons = sem("sem_cons")  # DVE consumed an xt tile (+1, load order)
        sem_dve = sem("sem_dve")    # DVE stats milestones
        sem_act = sem("sem_act")    # ACT stats milestones
        sem_pe = sem("sem_pe")      # PE matmul groups

        # Planned sem values after each named op, precomputed so any engine
        # stream can reference any other's milestones regardless of emission
        # order. Emitters assert against these as they bump.
        plan = {"memset": 1}
        for s in range(NS):
            # DVE: memset, [reduces s0] gcol0, [3 reduces s1] stats0 (9),
            # [reduces s1 rest] gcol1, stats1 (9), adds
            base = 2 + 10 * s  # dve count at gcol{s}
            plan[f"gcol{s}"] = base
            plan[f"z12_{s}"] = base + 1
            plan[f"s12_{s}"] = base + 2
            plan[f"u1m_{s}"] = base + 4
            plan[f"u0_{s}"] = base + 5
            plan[f"apr_{s}"] = base + 6
            plan[f"ppr_{s}"] = base + 7
            plan[f"zi_{s}"] = base + 9
            plan[f"fi_ev_{s}"] = 7 * s + 2
            plan[f"r2_{s}"] = 7 * s + 3
            plan[f"amp_{s}"] = 7 * s + 4
            plan[f"sinp_{s}"] = 7 * s + 6
            plan[f"xi_ev_{s}"] = 7 * s + 7
            # PE groups: fwd0=1, inv0=2, fwd1=3, inv1=4
            plan[f"fwd_{s}"] = 2 * s + 1
            plan[f"inv_{s}"] = 2 * s + 2

        dve_v = {"n": 0}
        act_v = {"n": 0}

        with nc.Block() as block:

            @block.vector
            def _(dve):
                nv = dve_v

                def bump(tag=None):
                    nv["n"] += 1
                    if tag:
                        assert plan[tag] == nv["n"], (tag, plan[tag], nv["n"])
                    return nv["n"]

                nc.vector.memset(halfpi[:], float(np.pi / 2)).then_inc(sem_dve, 1)
                bump("memset")

                def reduce_tile(s, r):
                    j = s * tps + r
                    cg, h = divmod(r, nhalf)
                    dve.wait_ge(ld_slot[j % nb_in], 16 * (j // nb_in + 1))
                    nc.vector.reduce_sum(
                        gsum[s][:, cg, h:h + 1], xt[j % nb_in][:],
                        axis=mybir.AxisListType.X,
                    ).then_inc(sem_cons, 1)

                def gcol_reduce(s):
                    dve.wait_ge(sem_cons, (s + 1) * tps)  # own gsum writes done
                    nc.vector.reduce_sum(
                        gcol[s][:], gsum[s][:], axis=mybir.AxisListType.X
                    ).then_inc(sem_dve, 1)
                    bump(f"gcol{s}")

                def stats_dve(s):
                    dve.wait_ge(sem_pe, plan[f"fwd_{s}"])
                    if s == 0:
                        dve.wait_ge(sem_cst, 16 * n_const)  # b_sb resident
                    nc.vector.tensor_add(
                        z12[s][:], fwd_ps[s][:, 2:4, :], b_sb[:]
                    ).then_inc(sem_dve, 1)
                    bump(f"z12_{s}")
                    # leaky_relu(z) = z + 0.99*relu(-z)
                    dve.wait_ge(sem_act, plan[f"r2_{s}"])
                    dve.wait_ge(sem_dve, plan[f"z12_{s}"])  # self RAW
                    nc.vector.scalar_tensor_tensor(
                        out=s12[s][:], in0=r2[s][:], scalar=0.99, in1=z12[s][:],
                        op0=mybir.AluOpType.mult, op1=mybir.AluOpType.add,
                    ).then_inc(sem_dve, 1)
                    bump(f"s12_{s}")
                    dve.wait_ge(sem_act, plan[f"fi_ev_{s}"])
                    nc.vector.tensor_mul(u0[s][:], fr[s][:], fr[s][:]).then_inc(
                        sem_dve, 1
                    )
                    bump()
                    nc.vector.tensor_mul(u1[s][:], fi[s][:], fi[s][:]).then_inc(
                        sem_dve, 1
                    )
                    bump(f"u1m_{s}")
                    dve.wait_ge(sem_dve, plan[f"u1m_{s}"])  # self RAW u0/u1
                    nc.vector.tensor_add(u0[s][:], u0[s][:], u1[s][:]).then_inc(
                        sem_dve, 1
                    )
                    bump(f"u0_{s}")
                    dve.wait_ge(sem_act, plan[f"amp_{s}"])
                    dve.wait_ge(sem_dve, plan[f"s12_{s}"])  # self RAW
                    nc.vector.tensor_mul(apr[s][:], s12[s][:, 0, :], amp[s][:]).then_inc(
                        sem_dve, 1
                    )
                    bump(f"apr_{s}")
                    nc.vector.tensor_mul(
                        ppr[s][:], s12[s][:, 1, :], fi[s][:]
                    ).then_inc(sem_dve, 1)
                    bump(f"ppr_{s}")
                    dve.wait_ge(sem_act, plan[f"sinp_{s}"])
                    dve.wait_ge(sem_dve, plan[f"apr_{s}"])  # self RAW
                    nc.vector.tensor_mul(zr[s][:], apr[s][:], cosp[s][:]).then_inc(
                        sem_dve, 1
                    )
                    bump()
                    nc.vector.tensor_mul(zi[s][:], apr[s][:], sinp[s][:]).then_inc(
                        sem_dve, 1
                    )
                    bump(f"zi_{s}")
                # ---- emission: interleave s0 stats into s1 reduces so the
                # xi_0 chain completes while pass-1 s1 streams ----
                for r in range(tps):
                    reduce_tile(0, r)
                gcol_reduce(0)
                ilv = min(3, tps)
                for r in range(ilv):
                    reduce_tile(1, r)
                stats_dve(0)
                for r in range(ilv, tps):
                    reduce_tile(1, r)
                gcol_reduce(1)
                stats_dve(1)

                # ---- pass 2 adds ----
                for m in range(n_x):
                    s, r = divmod(m, tps)
                    cg = r // nhalf
                    j = n_x + m  # global load index
                    if r == 0:
                        dve.wait_ge(sem_act, plan[f"xi_ev_{s}"])
                    dve.wait_ge(ld_slot[j % nb_in], 16 * (j // nb_in + 1))
                    if m >= nb_out:
                        dve.wait_ge(
                            st_slot[m % nb_out], 16 * ((m - nb_out) // nb_out + 1)
                        )
                    nc.vector.tensor_scalar_add(
                        yt[m % nb_out][:], xt[j % nb_in][:], xi[s][:, cg:cg + 1]
                    ).then_inc(sem_cons, 1)

            @block.scalar
            def _(act):
                nv = act_v

                def bump(tag=None):
                    nv["n"] += 1
                    if tag:
                        assert plan[tag] == nv["n"], (tag, plan[tag], nv["n"])
                    return nv["n"]

                act.wait_ge(sem_dve, plan["memset"])
                for s in range(NS):
                    act.wait_ge(sem_pe, plan[f"fwd_{s}"])
                    nc.scalar.mul(fr[s][:], fwd_ps[s][:, 0, :], 1.0 / hw)
                    bump()
                    nc.scalar.mul(fi[s][:], fwd_ps[s][:, 1, :], 1.0 / hw).then_inc(
                        sem_act, 2
                    )
                    bump(f"fi_ev_{s}")
                    act.wait_ge(sem_dve, plan[f"z12_{s}"])
                    nc.scalar.activation(
                        r2[s][:], z12[s][:], _AF.Relu, scale=-1.0
                    ).then_inc(sem_act, 1)
                    bump(f"r2_{s}")
                    act.wait_ge(sem_dve, plan[f"u0_{s}"])
                    nc.scalar.activation(amp[s][:], u0[s][:], _AF.Sqrt).then_inc(
                        sem_act, 1
                    )
                    bump(f"amp_{s}")
                    act.wait_ge(sem_dve, plan[f"ppr_{s}"])
                    nc.scalar.activation(
                        cosp[s][:], ppr[s][:], _AF.Sin, bias=halfpi[:]
                    )
                    bump()
                    nc.scalar.activation(sinp[s][:], ppr[s][:], _AF.Sin).then_inc(
                        sem_act, 2
                    )
                    bump(f"sinp_{s}")
                    act.wait_ge(sem_pe, plan[f"inv_{s}"])  # inverse mm of s done
                    nc.scalar.mul(xi[s][:], xi_ps[s][:], 1.0 / C).then_inc(
                        sem_act, 1
                    )
                    bump(f"xi_ev_{s}")

            @block.tensor
            def _(pe):
                pe.wait_ge(sem_cst, 16 * n_const)  # consts resident
                for s in range(NS):
                    # fwd s then inv s, so xi_0 is ready before sample-1
                    # stats (pass-2 s0 adds unblock as early as possible)
                    pe.wait_ge(sem_dve, plan[f"gcol{s}"])
                    last = None
                    for t, mat in enumerate((cos_sb, sin_sb, w1_sb, w2_sb)):
                        for kg in range(G):
                            for cg in range(G):
                                last = nc.tensor.matmul(
                                    fwd_ps[s][:, t, kg:kg + 1],
                                    mat[:, cg, kg * P:(kg + 1) * P],
                                    gcol[s][:, cg:cg + 1],
                                    start=(cg == 0),
                                    stop=(cg == G - 1),
                                )
                    last.then_inc(sem_pe, 1)  # fwd_s = 2s+1
                    pe.wait_ge(sem_dve, plan[f"zi_{s}"])
                    last = None
                    for cg in range(G):
                        for kg in range(G):
                            nc.tensor.matmul(
                                xi_ps[s][:, cg:cg + 1],
                                cos_sb[:, kg, cg * P:(cg + 1) * P],
                                zr[s][:, kg:kg + 1],
                                start=(kg == 0),
                                stop=False,
                            )
                            last = nc.tensor.matmul(
                                xi_ps[s][:, cg:cg + 1],
                                sin_sb[:, kg, cg * P:(cg + 1) * P],
                                zi[s][:, kg:kg + 1],
                                start=False,
                                stop=(kg == G - 1),
                            )
                    last.then_inc(sem_pe, 1)  # inv_s = 2s+2

            @block.sync
            def _(sp):
                for dram, sbuf in (
                    (cos_d, cos_sb), (sin_d, sin_sb), (w1_d, w1_sb),
                    (w2_d, w2_sb), (b_d, b_sb),
                ):
                    sp.dma_start(out=sbuf[:], in_=dram[:]).then_inc(sem_cst, 16)
                for j in range(2 * n_x):
                    if j >= nb_in:
                        sp.wait_ge(sem_cons, j - nb_in + 1)
                    sp.dma_start(
                        out=xt[j % nb_in][:], in_=x_ap(x_in, j % n_x)
                    ).then_inc(ld_slot[j % nb_in], 16)

            @block.gpsimd
            def _(gp):
                for m in range(n_x):
                    gp.wait_ge(sem_cons, n_x + m + 1)
                    gp.dma_start(
                        out=x_ap(x_out, m), in_=yt[m % nb_out][:]
                    ).then_inc(st_slot[m % nb_out], 16)

    return nc


_NC_CACHE = None


def _get_program():
    global _NC_CACHE
    if _NC_CACHE is None:
        _NC_CACHE = _build_program()
    return _NC_CACHE


def _host_constants():
    idx = np.arange(C)
    th = (2.0 * np.pi / C) * np.outer(idx, idx)
    cosm = np.cos(th).astype(np.float32)
    sinn = (-np.sin(th)).astype(np.float32)
    # [p, g, k] layout with row c = g*128+p
    to_pgk = lambda m: np.ascontiguousarray(m.reshape(G, P, C).transpose(1, 0, 2))
    return to_pgk(cosm), to_pgk(sinn)


_CONSTS_CACHE = None


def make_in_maps(inputs, hw=HW):
    """Shard + preprocess inputs into 8 per-core input maps."""
    global _CONSTS_CACHE
    if _CONSTS_CACHE is None:
        _CONSTS_CACHE = _host_constants()
    cos_pgk, sin_pgk = _CONSTS_CACHE

    x = np.ascontiguousarray(inputs["x"], dtype=np.float32)
    W1 = np.asarray(inputs["W1"], dtype=np.float32)
    W2 = np.asarray(inputs["W2"], dtype=np.float32)
    b1 = np.asarray(inputs["b1"], dtype=np.float32)
    b2 = np.asarray(inputs["b2"], dtype=np.float32)

    # fold the 1/HW mean normalization into the linear-layer weights
    w1t = np.ascontiguousarray(
        (W1.T / hw).reshape(G, P, C).transpose(1, 0, 2), dtype=np.float32
    )
    w2t = np.ascontiguousarray(
        (W2.T / hw).reshape(G, P, C).transpose(1, 0, 2), dtype=np.float32
    )
    bvec = np.ascontiguousarray(
        np.stack([b1.reshape(G, P), b2.reshape(G, P)]).transpose(2, 0, 1),
        dtype=np.float32,
    )  # [P, 2, G]

    xs = x.reshape(NCORES, NS, C, hw)
    return [
        {
            "x": xs[i],
            "cosm": cos_pgk,
            "sinn": sin_pgk,
            "w1t": w1t,
            "w2t": w2t,
            "bvec": bvec,
        }
        for i in range(NCORES)
    ]


def _run(inputs, trace=False, trace_kwargs=None):
    in_maps = make_in_maps(inputs)
    nc = _get_program()
    res = run_bass_kernel_spmd(
        nc,
        in_maps,
        list(range(NCORES)),
        trace=trace,
        **(trace_kwargs or {}),
    )
    out = np.stack([r["out"] for r in res.results])
    return out.reshape(N, C, H, W).astype(np.float32), res


def kernel(**inputs) -> np.ndarray:
    out, _ = _run(inputs, trace=False)
    return out
